# revision 1
# baseline (speedup 1.0000x reference)
"""Trainium2 Bass kernel for nn_Conv1d_NN (kNN + strided conv).

Math (per batch b):
    dist[t,s]  = ||x[:,t]||^2 + ||x[:,s]||^2 - 2 x[:,t].x[:,s]
    idx[t,:]   = top-8 smallest dist (self first), sorted ascending
    out[o,t]   = sum_{j,c} w[o,c,j] * x[c, idx[t,j]] + b[o]

Device strategy (data-parallel, 2 batches per core on 8 cores):
  - score[t,s] = 2 dot - ||x_s||^2 (row-constant shift of -dist preserves
    per-row ranking) via one K=65 fp32 matmul: lhsT=(x;1), rhs=(2x;-norm).
  - DVE max/max_index -> top-8 values + column indices per token
    (row tiles are strided: tile rt = tokens {q*16+rt}).
  - y[t,(j,o)] = sum_c x[c,t] w[o,c,j] + b[o]/8 via one K=65 matmul per
    tile against a [65, 512] weight block (ones row adds bias/8).
  - Outputs: y (all taps, all tokens) and the top-8 index table.

The final rank-indexed 8-way sum runs on the host: this container's
runtime has no working data-dependent DMA (HIPI gpsimd ucode excluded,
DynamicAP indirect DMA generates broken descriptors), so the O(T*K*C)
permutation+sum is applied to the device-computed y/idx tensors host-side.
All matmul FLOPs (distance matrix + conv) and the top-k run on device.
"""

import sys
import numpy as np

if "/opt/trn_rl_repo" not in sys.path:
    sys.path.insert(0, "/opt/trn_rl_repo")

B, C, T, K, OUT_C = 16, 64, 2048, 8, 64
NCORES = 8
BPC = B // NCORES  # batches per core
RT = T // 128      # 16 row tiles of 128 tokens
NF = T // 512      # 4 column chunks of 512

_CACHE = {}


def build_nc():
    import concourse.bacc as bacc
    import concourse.tile as tile
    import concourse.mybir as mybir

    dt = mybir.dt
    f32 = dt.float32
    Copy = mybir.ActivationFunctionType.Copy

    nc = bacc.Bacc(
        "TRN2", target_bir_lowering=False, debug=False, num_devices=NCORES
    )
    x_d = nc.dram_tensor("x", [BPC, C, T], f32, kind="ExternalInput").ap()
    wall_d = nc.dram_tensor("wall", [C + 1, K * OUT_C], f32, kind="ExternalInput").ap()
    y_d = nc.dram_tensor("yout", [BPC, K, T, OUT_C], f32, kind="ExternalOutput").ap()
    gi_d = nc.dram_tensor("gidx", [BPC, 128, 128], dt.uint16, kind="ExternalOutput").ap()

    with tile.TileContext(nc) as tc:
        with (
            tc.tile_pool(name="const", bufs=1) as constp,
            tc.tile_pool(name="xio", bufs=2) as xio,
            tc.tile_pool(name="scoresp", bufs=3) as scp,
            tc.tile_pool(name="small", bufs=2) as smp,
            tc.tile_pool(name="yio", bufs=3) as yp,
            tc.tile_pool(name="pd", bufs=6, space="PSUM") as pdp,
            tc.tile_pool(name="py", bufs=2, space="PSUM") as pyp,
        ):
            wall_sb = constp.tile([C + 1, K * OUT_C], f32)
            nc.sync.dma_start(wall_sb[:], wall_d[:])
            ones_sb = constp.tile([C, 1], f32)
            nc.gpsimd.memset(ones_sb[:], 1.0)

            for b in range(BPC):
                # ---- load x, build lhsT (x; 1) and rhs (2x; -norm) ----
                xlhs = xio.tile([C + 1, T], f32, tag="xlhs", name=f"xlhs{b}")
                nc.sync.dma_start(xlhs[0:C, :], x_d[b])
                nc.gpsimd.memset(xlhs[C : C + 1, :], 1.0)

                xsq = xio.tile([C, T], f32, tag="xsq", name=f"xsq{b}")
                nc.scalar.square(xsq[:], xlhs[0:C, :])

                xrhs = xio.tile([C + 1, T], f32, tag="xrhs", name=f"xrhs{b}")
                nc.scalar.activation(xrhs[0:C, :], xlhs[0:C, :], Copy, scale=2.0)
                for nf in range(NF):
                    pn = pyp.tile([1, 512], f32, tag="ps", name=f"pn{b}_{nf}")
                    nc.tensor.matmul(
                        pn[:], ones_sb[:], xsq[:, nf * 512 : (nf + 1) * 512]
                    )
                    nc.scalar.activation(
                        xrhs[C : C + 1, nf * 512 : (nf + 1) * 512],
                        pn[:],
                        Copy,
                        scale=-1.0,
                    )

                # row tile rt holds tokens t = q*16 + rt (strided slices)
                xl_t = xlhs.rearrange("c (q r) -> c r q", r=RT)
                yw = y_d[b].rearrange("j (q r) o -> r q j o", r=RT)

                # gall[q, j*16+rt] = idx of token q*16+rt, tap j
                gall = smp.tile([128, 128], dt.uint16, tag="gall", name=f"gall{b}")
                gall_v = gall.rearrange("q (j rt) -> q rt j", rt=RT)

                for rt in range(RT):
                    # contiguous copy of the strided token-tile for fast
                    # PE weight streaming
                    xtile = yp.tile([C + 1, 128], f32, tag="xtile", name=f"xt{b}_{rt}")
                    nc.scalar.copy(xtile[:], xl_t[:, rt, :])
                    scores = scp.tile([128, T], f32, tag="scores", name=f"sc{b}_{rt}")
                    for nf in range(NF):
                        pd = pdp.tile([128, 512], f32, tag="pd", name=f"pd{b}_{rt}_{nf}")
                        nc.tensor.matmul(
                            pd[:],
                            xtile[:],
                            xrhs[:, nf * 512 : (nf + 1) * 512],
                        )
                        nc.scalar.copy(scores[:, nf * 512 : (nf + 1) * 512], pd[:])
                    vals = smp.tile([128, 8], f32, tag="vals", name=f"v{b}_{rt}")
                    nc.vector.max(vals[:], scores[:])
                    nc.vector.max_index(gall_v[:, rt, :], vals[:], scores[:])

                    py = pyp.tile([128, 512], f32, tag="ps", name=f"py{b}_{rt}")
                    nc.tensor.matmul(py[:], xtile[:], wall_sb[:])
                    ysb = yp.tile([128, 512], f32, tag="ysb", name=f"y{b}_{rt}")
                    nc.scalar.copy(ysb[:], py[:])
                    nc.sync.dma_start(yw[rt], ysb.rearrange("p (j o) -> p j o", o=OUT_C))

                nc.sync.dma_start(gi_d[b], gall[:])

    nc.compile()
    return nc


def _get_nc():
    if "nc" not in _CACHE:
        _CACHE["nc"] = build_nc()
    return _CACHE["nc"]


def host_inputs(x, w, b):
    """Per-core input maps from full inputs."""
    x = np.asarray(x, dtype=np.float32)
    w = np.asarray(w, dtype=np.float32)
    b = np.asarray(b, dtype=np.float32)
    wall = np.empty((C + 1, K * OUT_C), np.float32)
    wall[:C] = w.transpose(1, 2, 0).reshape(C, K * OUT_C)  # [c, j*64+o]
    wall[C] = np.tile(b / K, K)  # ones row adds b/8 per tap
    return [
        {
            "x": np.ascontiguousarray(x[i * BPC : (i + 1) * BPC]),
            "wall": wall,
        }
        for i in range(NCORES)
    ]


def kernel(x, w, b):
    from concourse.bass_utils import run_bass_kernel_spmd

    nc = _get_nc()
    in_maps = host_inputs(x, w, b)
    res = run_bass_kernel_spmd(nc, in_maps, list(range(NCORES)))

    out = np.empty((B, OUT_C, T), np.float32)
    jj = np.arange(K, dtype=np.int64)[None, :]
    for i in range(NCORES):
        yv = res.results[i]["yout"]    # [BPC, K, T, OUT_C]
        gi = res.results[i]["gidx"]    # [BPC, 128, 128] u16
        for bb in range(BPC):
            # idx[t, j] with t = q*16 + rt stored at gall[q, j*16+rt]
            g = gi[bb].reshape(128, K, RT)          # [q, j, rt]
            idx = g.transpose(0, 2, 1).reshape(T, K).astype(np.int64)
            gathered = yv[bb][jj, idx, :]           # [T, K, OUT_C]
            out[i * BPC + bb] = gathered.sum(1).T
    return out.astype(np.float32)



# revision 2
# speedup vs baseline: 1.5844x; 1.5844x over previous
"""Trainium2 Bass kernel for nn_Conv1d_NN (kNN + strided conv).

Math (per batch b):
    dist[t,s]  = ||x[:,t]||^2 + ||x[:,s]||^2 - 2 x[:,t].x[:,s]
    idx[t,:]   = top-8 smallest dist (self first), sorted ascending
    out[o,t]   = sum_{j,c} w[o,c,j] * x[c, idx[t,j]] + b[o]

Device strategy (data-parallel, 2 batches per core on 8 cores):
  - score[t,s] = 2 dot - ||x_s||^2 (row-constant shift of -dist preserves
    per-row ranking), computed at ~fp32 precision with PAIRED bf16
    matmuls: x = xh + xl (two bf16 planes, host-split), then
        MM1: [xh;xl]^T [2xh;2xl]  (xh.2xh + xl.2xl)
        MM2: [xh;xl]^T [2xl;2xh]  (xh.2xl + xl.2xh)
        MM3: ones3^T  [-nhi;-nmid;-nlo]  (norm as 3 bf16 rows)
    accumulated in one fp32 PSUM bank per 512-chunk. bf16 streams at
    1 cycle/row vs fp32's 4, so this is ~2x cheaper than one fp32 MM
    and ~fp32-accurate (dropped residual ~2^-18, ~10 wrong neighbor
    indices out of 262144 -> rel err ~8e-3, tolerance 2e-2).
  - DVE max/max_index -> top-8 values + column indices per token
    (exact fp32 compare; token tiles are CONTIGUOUS 128-token slices).
  - y[t,(j,o)] = sum_c x[c,t] w[o,c,j] via the same [xh;xl] lhsT against
    a [128,512] bf16 weight block (w rows duplicated in both planes).
    Bias is added on the host during the gather.
  - Outputs: y (all taps, all tokens) and the top-8 index table.

The final rank-indexed 8-way gather+sum runs on the host: this
container's runtime has no working data-dependent DMA (HIPI gpsimd
ucode excluded, DynamicAP indirect DMA generates broken descriptors),
so the O(T*K*C) permutation+sum is applied to the device-computed
y/idx tensors host-side. All matmul FLOPs (distance matrix + conv) and
the top-k run on device.
"""

import sys
import numpy as np

if "/opt/trn_rl_repo" not in sys.path:
    sys.path.insert(0, "/opt/trn_rl_repo")

B, C, T, K, OUT_C = 16, 64, 2048, 8, 64
NCORES = 8
BPC = B // NCORES  # batches per core
RT = T // 128      # 16 row tiles of 128 tokens
NF = T // 512      # 4 column chunks of 512

_CACHE = {}


def build_nc():
    import concourse.bacc as bacc
    import concourse.tile as tile
    import concourse.mybir as mybir

    dt = mybir.dt
    f32 = dt.float32
    bf16 = dt.bfloat16

    nc = bacc.Bacc(
        "TRN2", target_bir_lowering=False, debug=False, num_devices=NCORES
    )
    xp_d = nc.dram_tensor("xpair", [BPC, 128, T], bf16, kind="ExternalInput").ap()
    ra_d = nc.dram_tensor("ra", [BPC, 128, T], bf16, kind="ExternalInput").ap()
    rb_d = nc.dram_tensor("rb", [BPC, 128, T], bf16, kind="ExternalInput").ap()
    nm_d = nc.dram_tensor("nrm", [BPC, 3, T], bf16, kind="ExternalInput").ap()
    wall_d = nc.dram_tensor("wall", [128, K * OUT_C], bf16, kind="ExternalInput").ap()
    y_d = nc.dram_tensor("yout", [BPC, T, K * OUT_C], f32, kind="ExternalOutput").ap()
    gi_d = nc.dram_tensor("gidx", [BPC, T, K], dt.uint16, kind="ExternalOutput").ap()

    with tile.TileContext(nc) as tc:
        with (
            tc.tile_pool(name="const", bufs=1) as constp,
            tc.tile_pool(name="xio", bufs=2) as xio,
            tc.tile_pool(name="scoresp", bufs=2) as scp,
            tc.tile_pool(name="small", bufs=3) as smp,
            tc.tile_pool(name="yio", bufs=3) as yp,
            tc.tile_pool(name="pd", bufs=4, space="PSUM") as pdp,
            tc.tile_pool(name="py", bufs=2, space="PSUM") as pyp,
        ):
            wall_sb = constp.tile([128, K * OUT_C], bf16)
            nc.sync.dma_start(wall_sb[:], wall_d[:])
            ones3 = constp.tile([3, 128], bf16)
            nc.gpsimd.memset(ones3[:], 1.0)

            for b in range(BPC):
                xp = xio.tile([128, T], bf16, tag="xp", name=f"xp{b}")
                ra = xio.tile([128, T], bf16, tag="ra", name=f"ra{b}")
                rb = xio.tile([128, T], bf16, tag="rb", name=f"rb{b}")
                nm = xio.tile([3, T], bf16, tag="nm", name=f"nm{b}")
                nc.sync.dma_start(xp[:], xp_d[b])
                nc.sync.dma_start(ra[:], ra_d[b])
                nc.sync.dma_start(rb[:], rb_d[b])
                nc.sync.dma_start(nm[:], nm_d[b])

                for rt in range(RT):
                    lhs = xp[:, rt * 128 : (rt + 1) * 128]
                    scores = scp.tile([128, T], f32, tag="scores", name=f"sc{b}_{rt}")
                    for nf in range(NF):
                        cs = slice(nf * 512, (nf + 1) * 512)
                        pd = pdp.tile([128, 512], f32, tag="pd", name=f"pd{b}_{rt}_{nf}")
                        nc.tensor.matmul(pd[:], lhs, ra[:, cs], start=True, stop=False)
                        nc.tensor.matmul(pd[:], lhs, rb[:, cs], start=False, stop=False)
                        nc.tensor.matmul(
                            pd[:], ones3[:], nm[:, cs], start=False, stop=True
                        )
                        nc.scalar.copy(scores[:, cs], pd[:])

                    vals = smp.tile([128, 8], f32, tag="vals", name=f"v{b}_{rt}")
                    nc.vector.max(vals[:], scores[:])
                    gall = smp.tile([128, 8], dt.uint16, tag="gall", name=f"g{b}_{rt}")
                    nc.vector.max_index(gall[:], vals[:], scores[:])
                    nc.sync.dma_start(gi_d[b, rt * 128 : (rt + 1) * 128, :], gall[:])

                    py = pyp.tile([128, 512], f32, tag="py", name=f"py{b}_{rt}")
                    nc.tensor.matmul(py[:], lhs, wall_sb[:])
                    ysb = yp.tile([128, 512], f32, tag="ysb", name=f"y{b}_{rt}")
                    nc.scalar.copy(ysb[:], py[:])
                    nc.sync.dma_start(y_d[b, rt * 128 : (rt + 1) * 128, :], ysb[:])

    nc.compile()
    return nc


def _get_nc():
    if "nc" not in _CACHE:
        _CACHE["nc"] = build_nc()
    return _CACHE["nc"]


def host_inputs(x, w, b):
    """Per-core input maps from full inputs."""
    import ml_dtypes

    bf = ml_dtypes.bfloat16
    x = np.asarray(x, dtype=np.float32)
    w = np.asarray(w, dtype=np.float32)
    b = np.asarray(b, dtype=np.float32)

    xh = x.astype(bf).astype(np.float32)
    xl = (x - xh).astype(bf).astype(np.float32)
    norm = (x.astype(np.float64) ** 2).sum(axis=1).astype(np.float32)  # [B, T]
    nhi = norm.astype(bf).astype(np.float32)
    nmid = (norm - nhi).astype(bf).astype(np.float32)
    nlo = (norm - nhi - nmid).astype(bf).astype(np.float32)

    xpair = np.concatenate([xh, xl], axis=1).astype(bf)           # [B, 128, T]
    ra = np.concatenate([2 * xh, 2 * xl], axis=1).astype(bf)      # [B, 128, T]
    rb = np.concatenate([2 * xl, 2 * xh], axis=1).astype(bf)      # [B, 128, T]
    nrm = np.stack([-nhi, -nmid, -nlo], axis=1).astype(bf)        # [B, 3, T]

    wr = w.transpose(1, 2, 0).reshape(C, K * OUT_C)               # [c, j*64+o]
    wall = np.concatenate([wr, wr], axis=0).astype(bf)            # [128, 512]

    return [
        {
            "xpair": np.ascontiguousarray(xpair[i * BPC : (i + 1) * BPC]),
            "ra": np.ascontiguousarray(ra[i * BPC : (i + 1) * BPC]),
            "rb": np.ascontiguousarray(rb[i * BPC : (i + 1) * BPC]),
            "nrm": np.ascontiguousarray(nrm[i * BPC : (i + 1) * BPC]),
            "wall": wall,
        }
        for i in range(NCORES)
    ]


def kernel(x, w, b):
    from concourse.bass_utils import run_bass_kernel_spmd

    nc = _get_nc()
    in_maps = host_inputs(x, w, b)
    res = run_bass_kernel_spmd(nc, in_maps, list(range(NCORES)))

    b32 = np.asarray(b, dtype=np.float32)
    out = np.empty((B, OUT_C, T), np.float32)
    jj = np.arange(K, dtype=np.int64)[None, :]
    for i in range(NCORES):
        yv = res.results[i]["yout"]    # [BPC, T, K*OUT_C]
        gi = res.results[i]["gidx"]    # [BPC, T, K] u16
        for bb in range(BPC):
            idx = gi[bb].astype(np.int64)                 # [T, K]
            yr = yv[bb].reshape(T, K, OUT_C)              # [s, j, o]
            gathered = yr[idx, jj, :]                     # [T, K, OUT_C]
            out[i * BPC + bb] = gathered.sum(1).T + b32[:, None]
    return out.astype(np.float32)
